# revision 13
# baseline (speedup 1.0000x reference)
"""Trainium2 Bass kernel for BaseViTSelfAttention (cross/self attention, 16 heads).

Computation (per batch element b):
    q = hidden @ Wq.T            [1024, 1024]
    ctx = concat(hidden, context)  [1280, 1024]
    k = ctx @ Wk.T; v = ctx @ Wv.T
    out = softmax(q_h @ k_h.T / 8) @ v_h   per 64-dim head, reassembled

Sharding: batch-parallel, one batch element per NeuronCore (8 cores).
Host-side prep (numpy, layout + fp16 cast): transpose weights to [di, do]
and build ctxT = concat(hidden, context).transpose -> [D, NK] per batch so
the contraction dim lands on SBUF partitions.

Structure: V projection first, then one fused loop over head pairs that
computes the K/Q projection slices for that pair and immediately runs
attention on them.  Scores for the head pair run as concurrent row-tiled
matmuls at partition offsets 0/64 (measured ~3x on HW vs sequential).
Softmax denominators come for free from a ones-column appended to v.
All matmuls run in fp16 with fp32 PSUM accumulation (separate LDWEIGHTS
hides the weight-load; fp32r self-loading matmuls pay ~25% extra).  The
softmax operates on scores/8 ~ N(0,1), so fp16 rounding of q/k/probs/v
contributes only ~1e-3 relative error overall.

vs the earlier checkpoint: the output is stored as fp16 (halves store
traffic; host upcasts), ctxT and v are double-buffered so the next
repeat-loop iteration's input DMA and V projection overlap the current
tail, the output staging/transposes run in fp16 with all four transposes
landing in slots of one PSUM bank (no serialization on the staging tile),
and the next pair's K/Q projection groups are emitted between the pv
chains to fill the PE's exp-cadence stall windows (in-order queue); the
four nq-tile stores per chunk are fused into one 128KB DMA (1/4 the DGE
issue + semaphore overhead).
(Attempts that regressed, kept out: gpsimd partition_broadcast/all_reduce
normalize tails serialize ~2x on HW when interleaved with other engine
traffic; di-interleaved projection weight reuse and N=1024 matmuls are
rejected or slower -- see memory notes.)

Biases are all-zero for this problem spec and are ignored.
"""
import numpy as np

import concourse.bass as bass
import concourse.mybir as mybir
import concourse.tile as tile
from concourse import bacc
from concourse.bass import ds, ts
from concourse.bass_utils import run_bass_kernel_spmd

N_CORES = 8
P = 128
D = 1024          # model dim
NQ = 1024         # query length (hidden)
NK = 1280         # key/value length (hidden + context)
H = 16            # heads
DH = 64           # head dim
DT = D // P       # 8 contraction tiles
NKT = NK // P     # 10 nk tiles
SCALE = 1.0 / 8.0  # 1/sqrt(DH)
F32 = mybir.dt.float32
F32R = mybir.dt.float32r
F16 = mybir.dt.float16
F8 = mybir.dt.float8e4
DR = mybir.MatmulPerfMode.DoubleRow
EXP_SHIFT = -2.0  # keep exp(s/8-2) under fp8e4's +-240 saturation
NQC = 512         # nq chunk for attention
NCH = NQ // NQC   # 2 chunks


def emit(nc, tc, ctx_d, wq_d, wk_d, wv_d, out_d, repeat=1):
    with (
        tc.tile_pool(name="persist", bufs=1) as persist,
        tc.tile_pool(name="wp", bufs=24) as wp,
        tc.tile_pool(name="kqp", bufs=2) as kqp,
        tc.tile_pool(name="vp", bufs=2) as vp,
        tc.tile_pool(name="p2", bufs=2) as p2,
        tc.tile_pool(name="stg", bufs=4) as stg,
        tc.tile_pool(name="ctxp", bufs=2) as ctxp,
        tc.tile_pool(name="psp", bufs=2, space="PSUM") as psp,
        tc.tile_pool(name="pss", bufs=1, space="PSUM") as pss,
        tc.tile_pool(name="pso", bufs=1, space="PSUM") as pso,
        tc.tile_pool(name="pst", bufs=1, space="PSUM") as pst,
    ):
        from concourse.masks import make_identity
        ident = persist.tile([P, P], F16, tag="ident")
        make_identity(nc, ident[:])
        ebias = persist.tile([P, 1], F32, tag="ebias")
        nc.vector.memset(ebias[:], EXP_SHIFT)
        if repeat == 1:
            _emit_iter(nc, tc, vp, ctxp, wp, kqp, p2, stg, psp, pss,
                       pso, pst, ident, ebias, ctx_d, wq_d, wk_d, wv_d,
                       out_d)
        else:
            # hardware loop: used only for wall-clock timing builds
            with tc.For_i(0, repeat, 1):
                _emit_iter(nc, tc, vp, ctxp, wp, kqp, p2, stg, psp,
                           pss, pso, pst, ident, ebias, ctx_d, wq_d,
                           wk_d, wv_d, out_d)


def _emit_iter(nc, tc, vp, ctxp, wp, kqp, p2, stg, psp, pss, pso, pst,
               ident, ebias, ctx_d, wq_d, wk_d, wv_d, out_d):
    v = vp.tile([P, NKT, H, DH + 1], F16, tag="v")  # natural v + ones col
    nc.vector.memset(v[:, :, :, DH:DH + 1], 1.0)

    ctxT = ctxp.tile([P, DT, NK], F16, tag="ctxT")

    # DMA: ctxT (sync queue) || wk (gpsimd queue) land first so the pair-0
    # K/Q projection -- and with it the exp pipeline -- starts as early as
    # possible; wq next, wv last (the V projection is deferred filler).
    wk, wq, wv = [], [], []
    for t in range(DT):
        nc.sync.dma_start(ctxT[:, t, :], ctx_d[ts(t, P), :])
        wt = wp.tile([P, D], F16, tag="w", name=f"wk_{t}")
        nc.gpsimd.dma_start(wt[:], wk_d[ts(t, P), :])
        wk.append(wt)
    for t in range(DT):
        wt = wp.tile([P, D], F16, tag="w", name=f"wq_{t}")
        nc.sync.dma_start(wt[:], wq_d[ts(t, P), :])
        wq.append(wt)
    for t in range(DT):
        wt = wp.tile([P, D], F16, tag="w", name=f"wv_{t}")
        nc.gpsimd.dma_start(wt[:], wv_d[ts(t, P), :])
        wv.append(wt)

    def v_group(m, c0, w):
        # v[nk-tile m, heads c0/DH..] = sum_di ctxT[di,m].T @ WvT[di,c0:c0+w]
        def go():
            ps = psp.tile([P, 512], F32, tag="ps")
            for di in range(DT):
                nc.tensor.matmul(
                    ps[:, :w],
                    ctxT[:, di, ts(m, P)],
                    wv[di][:, ds(c0, w)],
                    start=(di == 0),
                    stop=(di == DT - 1),
                )
            nc.vector.tensor_copy(
                v[:, m, ds(c0 // DH, w // DH), 0:DH],
                ps[:, :w].rearrange("p (h d) -> p h d", h=w // DH),
            )
        return go

    def proj_groups(hp):
        kT = kqp.tile([P, NK], F16, tag="kT", name=f"kT_{hp}")
        qT = kqp.tile([P, NQ], F16, tag="qT", name=f"qT_{hp}")

        def k_group(c0, w):
            def go():
                ps = psp.tile([P, 512], F32, tag="ps")
                for di in range(DT):
                    nc.tensor.matmul(
                        ps[:, :w],
                        wk[di][:, ts(hp, P)],
                        ctxT[:, di, ds(c0, w)],
                        start=(di == 0),
                        stop=(di == DT - 1),
                    )
                nc.vector.tensor_copy(kT[:, ds(c0, w)], ps[:, :w])
            return go

        def q_group(c):
            def go():
                ps = psp.tile([P, 512], F32, tag="ps")
                for di in range(DT):
                    nc.tensor.matmul(
                        ps[:],
                        wq[di][:, ts(hp, P)],
                        ctxT[:, di, ds(c * 512, 512)],
                        start=(di == 0),
                        stop=(di == DT - 1),
                    )
                nc.vector.tensor_copy(qT[:, ds(c * 512, 512)], ps[:])
            return go

        groups = [k_group(0, 512), k_group(512, 512), k_group(1024, 256),
                  q_group(0), q_group(1)]
        return kT, qT, groups

    # ---- filler queue ----
    # All PE work that is off the exp critical path (V projection, later
    # pairs' K/Q projections) is queued and popped into the exp-cadence
    # stall windows of the in-order PE queue.  Forced drains before each
    # consumer guarantee emission-order correctness.
    filler = []
    state = {"drained": 0}

    def pop_filler():
        if state["drained"] < len(filler):
            filler[state["drained"]]()
            state["drained"] += 1

    def drain_to(k):
        while state["drained"] < min(k, len(filler)):
            pop_filler()

    def score_chunk(pair, kT, qT, c):
        # merged layout: et[:, t, hh, :] = exp of head pair[hh], nk-tile t.
        et = p2.tile([P, NKT, 2, NQC], F16, tag="expT")
        # scoresT[nk, nq]: head pair at partition offsets 0/64 emitted
        # interleaved -> concurrent row-tiled matmuls; BOTH heads' 2
        # nk-tiles share one 4-bank psum tile so each exp is a single
        # 2048-elem ACT instruction (halves the per-instr overhead count).
        for g in range(NKT // 2):
            pp = pss.tile([P, 2, 2, NQC], F32, tag="pss")
            for tt in range(2):
                for h in pair:
                    o = 64 * (h % 2)
                    nc.tensor.matmul(
                        pp[:, tt, h % 2, :],
                        kT[o:o + DH, ts(2 * g + tt, P)],
                        qT[o:o + DH, ds(c * NQC, NQC)],
                        start=True,
                        stop=True,
                    )
            nc.scalar.activation(
                et[:, ds(2 * g, 2), :, :], pp[:, :, :, :],
                mybir.ActivationFunctionType.Exp,
                scale=SCALE,
            )
            pop_filler()  # fill the exp-wait window
        return et

    # pair-0 K/Q first: the exp pipeline starts ~15us in instead of ~60us
    kT0, qT0, g0 = proj_groups(0)
    for g in g0:
        g()
    kqt = (kT0, qT0)

    # v heads 0-3 / 4-7 as F=256 quarters (early pv unblock), 8-15 as
    # F=512 halves
    filler += [v_group(m, 0, 256) for m in range(NKT)]
    kq_marks = [0]
    v_need = {}

    for hp in range(H // 2):
        pair = (2 * hp, 2 * hp + 1)
        kT, qT = kqt
        if hp + 1 < H // 2:
            nkT, nqT, nxt = proj_groups(hp + 1)
            filler.extend(nxt)
            kq_marks.append(len(filler))
            kqt = (nkT, nqT)
        if hp == 0:
            filler.extend(v_group(m, 256, 256) for m in range(NKT))
            v_need[2] = v_need[3] = len(filler)
        elif hp == 1:
            filler.extend(v_group(m, 512, 512) for m in range(NKT))
            for p in range(4, H // 2):
                v_need[p] = len(filler)
        v_need.setdefault(0, NKT)
        v_need.setdefault(1, NKT)

        drain_to(kq_marks[hp])
        for c in range(NCH):
            # one staging tile for all 4 nq-tiles of the chunk: a single
            # 128KB store replaces 4x32KB (same descriptors, 1/4 the DMA
            # issue + semaphore overhead on the DGE queues)
            otp = stg.tile([P, 4, 2, DH], F16, tag="outstg",
                           name=f"otp_{c}")
            et = score_chunk(pair, kT, qT, c)
            drain_to(v_need[hp])  # v coverage before the pv chains
            # outT_aug[65, nq] = sum_nk v_aug[nk, 65] * expT[nk, nq]
            for h in pair:
                po = pso.tile([DH + 1, NQC], F32, tag="pso")
                for t in range(NKT):
                    nc.tensor.matmul(
                        po[:],
                        v[:, t, h, :],
                        et[:, t, h % 2, :],
                        start=(t == 0),
                        stop=(t == NKT - 1),
                    )
                st = stg.tile([DH + 1, NQC], F16, tag="stage")
                nc.vector.tensor_copy(st[:], po[:])
                # all 4 transposes land in slots of ONE psum bank so no
                # transpose waits the previous slot's consumers; fp16
                # halves the PE transpose cost vs fp32
                # slots padded to DH+2 so each 2-byte slot start stays
                # 4-byte aligned (PSUM requirement)
                pt4 = pst.tile([P, 4, DH + 2], F16, tag="pst",
                               name=f"pt4_{h}")
                for j in range(NQC // P):
                    nc.tensor.transpose(
                        pt4[:, j, 0:DH + 1], st[:, ts(j, P)],
                        ident[:DH + 1, :DH + 1]
                    )
                rc4 = stg.tile([P, 4], F32, tag="recip")
                nc.vector.reciprocal(rc4[:], pt4[:, :, DH:DH + 1])
                for j in range(NQC // P):
                    nc.vector.tensor_scalar_mul(
                        otp[:, j, h % 2, :], pt4[:, j, 0:DH],
                        rc4[:, j:j + 1]
                    )
                pop_filler()  # fill pv-stall window
            eng = nc.gpsimd if c % 2 else nc.sync
            eng.dma_start(
                out_d[hp, ds(c * NQC, NQC), :].rearrange(
                    "(t p) d -> p t d", p=P
                ),
                otp[:],
            )
    drain_to(len(filler))


_CACHE = {}


def build(repeat=1):
    key = repeat
    if key in _CACHE:
        return _CACHE[key]
    nc = bacc.Bacc("TRN2", target_bir_lowering=False, debug=False,
                   num_devices=N_CORES)
    ctx_d = nc.dram_tensor("ctxT", [D, NK], F16, kind="ExternalInput")
    wq_d = nc.dram_tensor("wqT", [D, D], F16, kind="ExternalInput")
    wk_d = nc.dram_tensor("wkT", [D, D], F16, kind="ExternalInput")
    wv_d = nc.dram_tensor("wvT", [D, D], F16, kind="ExternalInput")
    out_d = nc.dram_tensor("out", [H // 2, NQ, 2 * DH], F16,
                           kind="ExternalOutput")
    with tile.TileContext(nc) as tc:
        emit(nc, tc, ctx_d, wq_d, wk_d, wv_d, out_d, repeat=repeat)
    nc.compile()
    _CACHE[key] = (nc, ctx_d, wq_d, wk_d, wv_d, out_d)
    return _CACHE[key]


def make_in_maps(hidden_states, context_states, Wq, Wk, Wv):
    ctxT = np.ascontiguousarray(
        np.concatenate([hidden_states, context_states], axis=1).transpose(0, 2, 1)
    ).astype(np.float16)
    wqT = np.ascontiguousarray(np.asarray(Wq).T).astype(np.float16)
    wkT = np.ascontiguousarray(np.asarray(Wk).T).astype(np.float16)
    wvT = np.ascontiguousarray(np.asarray(Wv).T).astype(np.float16)
    return [
        {"ctxT": ctxT[b], "wqT": wqT, "wkT": wkT, "wvT": wvT}
        for b in range(N_CORES)
    ]


def kernel(hidden_states, context_states, Wq, bq, Wk, bk, Wv, bv):
    # bq/bk/bv are zeros per the problem spec; not applied.
    nc = build(repeat=1)[0]
    in_maps = make_in_maps(hidden_states, context_states, Wq, Wk, Wv)
    res = run_bass_kernel_spmd(nc, in_maps, core_ids=list(range(N_CORES)))
    # device writes fp16 [H/2, NQ, 2*DH]; un-permute + upcast on host
    return np.stack(
        [
            res.results[b]["out"].transpose(1, 0, 2).reshape(NQ, D)
            .astype(np.float32)
            for b in range(N_CORES)
        ],
        axis=0,
    )

